# revision 3
# baseline (speedup 1.0000x reference)
"""Fallback/optimized variant: convs as explicit patchify + matmul (no lax.conv)."""

import functools

import numpy as np
import jax
import jax.numpy as jnp
from jax import lax

T, B, HW, HID, NA = 128, 64, 52, 512, 15
NDEV = 8
BL = B // NDEV


def _prep_weights(conv_w1, conv_w2, conv_w3, fc_w):
    # conv1: [32,3,8,8] -> [192,32] with row order (dy,dx,ry,rx,c); fold 1/255
    w = np.asarray(conv_w1)  # [o,c,ky,kx]
    w = w.transpose(2, 3, 1, 0)  # [ky,kx,c,o]
    w = w.reshape(2, 4, 2, 4, 3, 32)  # [dy,ry,dx,rx,c,o]
    w = w.transpose(0, 2, 1, 3, 4, 5)  # [dy,dx,ry,rx,c,o]
    w1m = (w.reshape(192, 32) / 255.0).astype(np.float32)

    # conv2: [64,32,4,4] -> [512,64], row order (dy,dx,ry,rx,c)
    w = np.asarray(conv_w2).transpose(2, 3, 1, 0)  # [ky,kx,c,o]
    w = w.reshape(2, 2, 2, 2, 32, 64).transpose(0, 2, 1, 3, 4, 5)
    w2m = w.reshape(512, 64).astype(np.float32)

    # conv3: [64,64,3,3] -> [576,64], row order (ky,kx,c)
    w = np.asarray(conv_w3).transpose(2, 3, 1, 0)
    w3m = w.reshape(576, 64).astype(np.float32)

    # fc expects flatten order (c,h,w); we produce (h,w,c) -> permute fc rows
    fw = np.asarray(fc_w)  # [576, HID], rows indexed c*9+h*3+w
    h, wq, c = np.meshgrid(np.arange(3), np.arange(3), np.arange(64), indexing="ij")
    src = (c * 9 + h * 3 + wq).reshape(576)
    fcm = fw[src].astype(np.float32)
    return w1m, w2m, w3m, fcm


def _forward_shard(x, done, h0, c0, action, W):
    N = T * BL
    # conv1: patchify 4x4 blocks
    z = x.reshape(N, 13, 4, 13, 4, 3)
    z = z.transpose(0, 1, 3, 2, 4, 5).reshape(N, 13, 13, 48)
    p = jnp.concatenate(
        [
            z[:, 0:12, 0:12], z[:, 0:12, 1:13],
            z[:, 1:13, 0:12], z[:, 1:13, 1:13],
        ],
        axis=-1,
    )  # [N,12,12,192] rows (dy,dx, blockfeat)
    y = jax.nn.relu(p.reshape(N * 144, 192) @ W["w1m"] + W["conv_b1"])  # [N*144,32]

    # conv2: input [N,12,12,32], stride2 k4 -> blocks 2x2
    z = y.reshape(N, 6, 2, 6, 2, 32).transpose(0, 1, 3, 2, 4, 5).reshape(N, 6, 6, 128)
    p = jnp.concatenate(
        [z[:, 0:5, 0:5], z[:, 0:5, 1:6], z[:, 1:6, 0:5], z[:, 1:6, 1:6]], axis=-1
    )  # [N,5,5,512]
    y = jax.nn.relu(p.reshape(N * 25, 512) @ W["w2m"] + W["conv_b2"])  # [N*25,64]

    # conv3: [N,5,5,64] k3 s1 -> im2col 9 shifts, row order (ky,kx,c)
    z = y.reshape(N, 5, 5, 64)
    p = jnp.concatenate(
        [z[:, ky : ky + 3, kx : kx + 3] for ky in range(3) for kx in range(3)],
        axis=-1,
    )  # [N,3,3,576]
    y = jax.nn.relu(p.reshape(N * 9, 576) @ W["w3m"] + W["conv_b3"])  # [N*9,64]

    feat = y.reshape(N, 576)  # (h,w,c) order; fc rows pre-permuted to match
    hidden = jax.nn.relu(feat @ W["fcm"] + W["fc_b"])  # [N,HID]

    seq = hidden.reshape(T, BL, HID)
    dseq = done.reshape(T, BL)
    w_ih_t = W["w_ih"].T
    w_hh_t = W["w_hh"].T
    b_lstm = W["b_lstm"]

    def step(carry, inp):
        h, c = carry
        xt, dt = inp
        m = (1.0 - dt)[:, None]
        h = h * m
        c = c * m
        gates = xt @ w_ih_t + h @ w_hh_t + b_lstm
        i, f, g, o = jnp.split(gates, 4, axis=-1)
        c = jax.nn.sigmoid(f) * c + jax.nn.sigmoid(i) * jnp.tanh(g)
        h = jax.nn.sigmoid(o) * jnp.tanh(c)
        return (h, c), h

    (hT, cT), hs = lax.scan(step, (h0, c0), (seq, dseq))
    feats = hs.reshape(N, HID)

    logits = feats @ W["actor_w"] + W["actor_b"]
    logp = jax.nn.log_softmax(logits)
    lp_a = jnp.take_along_axis(logp, action[:, None], axis=1)[:, 0]
    entropy = -jnp.sum(jnp.exp(logp) * logp, axis=-1)
    value = feats @ W["critic_w"] + W["critic_b"]
    return lp_a, entropy, value, hT, cT


@functools.partial(jax.pmap, axis_name="d", in_axes=(0, 0, 0, 0, 0, None))
def _pmapped(x, done, h0, c0, action, W):
    return _forward_shard(x, done, h0, c0, action, W)


def _shard_bt(a):
    a = a.reshape(T, NDEV, BL, *a.shape[1:])
    a = np.moveaxis(a, 1, 0)
    return np.ascontiguousarray(a.reshape(NDEV, T * BL, *a.shape[3:]))


def _unshard_bt(a):
    a = a.reshape(NDEV, T, BL, *a.shape[2:])
    a = np.moveaxis(a, 0, 1)
    return np.ascontiguousarray(a.reshape(T * B, *a.shape[3:]))


def kernel(
    x, done, h0, c0,
    conv_w1, conv_b1, conv_w2, conv_b2, conv_w3, conv_b3,
    fc_w, fc_b, w_ih, w_hh, b_lstm, actor_w, actor_b, critic_w, critic_b,
    action,
):
    x = np.asarray(x, dtype=np.float32)
    w1m, w2m, w3m, fcm = _prep_weights(conv_w1, conv_w2, conv_w3, fc_w)
    W = {
        "w1m": jnp.asarray(w1m), "conv_b1": jnp.asarray(np.asarray(conv_b1)),
        "w2m": jnp.asarray(w2m), "conv_b2": jnp.asarray(np.asarray(conv_b2)),
        "w3m": jnp.asarray(w3m), "conv_b3": jnp.asarray(np.asarray(conv_b3)),
        "fcm": jnp.asarray(fcm), "fc_b": jnp.asarray(np.asarray(fc_b)),
        "w_ih": jnp.asarray(np.asarray(w_ih)), "w_hh": jnp.asarray(np.asarray(w_hh)),
        "b_lstm": jnp.asarray(np.asarray(b_lstm)),
        "actor_w": jnp.asarray(np.asarray(actor_w)),
        "actor_b": jnp.asarray(np.asarray(actor_b)),
        "critic_w": jnp.asarray(np.asarray(critic_w)),
        "critic_b": jnp.asarray(np.asarray(critic_b)),
    }
    xs = _shard_bt(x)
    ds = _shard_bt(np.asarray(done, np.float32))
    acs = _shard_bt(np.asarray(action, np.int32))
    h0s = np.ascontiguousarray(np.asarray(h0, np.float32).reshape(NDEV, BL, HID))
    c0s = np.ascontiguousarray(np.asarray(c0, np.float32).reshape(NDEV, BL, HID))

    try:
        lp_a, entropy, value, hT, cT = _pmapped(xs, ds, h0s, c0s, acs, W)
        lp_a = np.asarray(lp_a)
    except Exception as e:
        # device compile/run failed: correct CPU fallback
        import sys
        print(f"kernel: device path failed ({type(e).__name__}); CPU fallback", file=sys.stderr)
        import jax as _jax
        cpu = _jax.devices("cpu")[0]
        with _jax.default_device(cpu):
            outs = [_forward_shard(jnp.asarray(xs[i]), jnp.asarray(ds[i]),
                                   jnp.asarray(h0s[i]), jnp.asarray(c0s[i]),
                                   jnp.asarray(acs[i]), W) for i in range(NDEV)]
        lp_a = np.stack([np.asarray(o[0]) for o in outs])
        entropy = np.stack([np.asarray(o[1]) for o in outs])
        value = np.stack([np.asarray(o[2]) for o in outs])
        hT = np.stack([np.asarray(o[3]) for o in outs])
        cT = np.stack([np.asarray(o[4]) for o in outs])
    return (
        _unshard_bt(np.asarray(lp_a)).astype(np.float32),
        _unshard_bt(np.asarray(entropy)).astype(np.float32),
        _unshard_bt(np.asarray(value)).astype(np.float32),
        np.asarray(hT).reshape(B, HID).astype(np.float32),
        np.asarray(cT).reshape(B, HID).astype(np.float32),
    )
